# revision 22
# baseline (speedup 1.0000x reference)
"""Causal self-attention (B=4, T=1024, C=1024, H=16) on 8 TRN2 NeuronCores.

Sharding: tensor-parallel over heads — 2 heads per core. x is replicated;
each core computes qkv for its heads, attention, and a partial output
projection (its heads' columns of w_out); the host sums the 8 partials.

Per-core dataflow (all matmuls fp32r):
  phase 1: qkvT[chan, tok] = wqkvT.T @ xT per 512-token group;
           q,k kept as [d, tok] (2 heads packed on 128 partitions);
           v PE-transposed to [tok, d] with a ones column appended.
  phase 2: per (batch, 512-query group, 128-key block):
           ST[key, query] = kT.T @ qT  (2 heads row-packed in the PE array)
           PT = exp(ST/8)  (causal: block skipping + additive mask on the
           diagonal 128x128 block; masked key>query entries become exp(-1e10)=0)
           OT[d+1, query] += v_aug.T @ PT  (row 64 accumulates the softmax
           denominator via the ones column)
           normalize: y = OT[0:64] * broadcast(1/OT[64])
  phase 3: out[tok, :] = yT.T @ woT, DMA'd PSUM->HBM directly.
"""

import sys
import types

import numpy as np

import concourse.bacc as bacc
import concourse.mybir as mybir
import concourse.tile as tile
from concourse.bass_utils import run_bass_kernel_spmd
from concourse.masks import make_identity

F32 = mybir.dt.float32
F32R = mybir.dt.float32r
Exp = mybir.ActivationFunctionType.Exp

P = 128
B = 4
T = 1024
C = 1024
N_HEAD = 16
DH = 64
BT = B * T           # 4096 tokens
NCO = C // P         # 8 contraction blocks
NTG = BT // 512      # 8 token groups of 512
QG_PER_B = T // 512  # 2 query groups per batch
N_CORES = 8
H_LOC = N_HEAD // N_CORES  # 2 local heads

MASK_VAL = -1e10
SCALE = 1.0 / np.sqrt(np.float32(DH))  # 0.125


def build_nc():
    nc = bacc.Bacc("TRN2", target_bir_lowering=False, debug=False)

    xT = nc.dram_tensor("xT", [C, BT], F32R, kind="ExternalInput")
    wq = nc.dram_tensor("wq", [C, 3 * P], F32R, kind="ExternalInput")
    wo = nc.dram_tensor("wo", [P, C], F32R, kind="ExternalInput")
    out = nc.dram_tensor("out", [BT, C], F32, kind="ExternalOutput")

    with tile.TileContext(nc) as tc:
        with (
            tc.tile_pool(name="consts", bufs=1) as consts,
            tc.tile_pool(name="xin", bufs=3) as xin,
            tc.tile_pool(name="vt", bufs=3) as vtp,
            tc.tile_pool(name="pt", bufs=8) as ptp,
            tc.tile_pool(name="ep", bufs=4) as epp,
            tc.tile_pool(name="outp", bufs=4) as outp,
            tc.tile_pool(name="ps_mm", bufs=2, space="PSUM") as ps_mm,
            tc.tile_pool(name="ps_st", bufs=4, space="PSUM") as ps_st,
            tc.tile_pool(name="ps_ot", bufs=2, space="PSUM") as ps_ot,
        ):
            # ---- constants / persistent tiles ----
            # first x tile's chunks go first so the PE can start ~asap;
            # weights interleave per contraction chunk
            x_first = xin.tile([P, NCO, 512], F32R, tag="x", name="x_first")
            w_sb = consts.tile([P, NCO, 3 * P], F32R)
            for co in range(NCO):
                nc.sync.dma_start(x_first[:, co, :], xT[co * P:(co + 1) * P, 0:512])
                nc.sync.dma_start(w_sb[:, co, :], wq[co * P:(co + 1) * P, :])
            wo_sb = consts.tile([P, C], F32R)

            qT_all = consts.tile([P, NTG, 512], F32R)
            kT_all = consts.tile([P, NTG, 512], F32R)
            # v with ones column at index DH (softmax denominator trick)
            v_aug = [
                consts.tile([P, BT // P, DH + 1], F32R, tag=f"v{h}", name=f"v{h}")
                for h in range(H_LOC)
            ]

            ident = consts.tile([P, P], F32)
            make_identity(nc, ident[:])
            # multiplicative causal mask for the diagonal 128x128 block of
            # PT[key, query]: valid iff key_row <= query_col (keep where
            # col - row >= 0). Applied to the exp output in SBUF, off the
            # ST-psum critical path.
            tri01 = consts.tile([P, P], F32)
            nc.gpsimd.memset(tri01[:], 1.0)
            nc.gpsimd.affine_select(
                out=tri01[:], in_=tri01[:],
                compare_op=mybir.AluOpType.is_ge, fill=0.0,
                base=0, pattern=[[1, P]], channel_multiplier=-1,
            )
            ones_f = consts.tile([P, BT // P], F32)
            nc.vector.memset(ones_f[:], 1.0)
            for h in range(H_LOC):
                nc.vector.tensor_copy(v_aug[h][:, :, DH], ones_f[:])

            # ---- phase 1: qkv projection per 512-token group ----
            for tg in range(NTG):
                if tg == 0:
                    x_tile = x_first
                else:
                    x_tile = xin.tile([P, NCO, 512], F32R, tag="x")
                    nc.sync.dma_start(
                        x_tile[:],
                        xT[:].rearrange("(a p) t -> p a t", p=P)[
                            :, :, tg * 512:(tg + 1) * 512
                        ],
                    )
                for cb in range(3):  # 0=q, 1=k, 2=v
                    ps = ps_mm.tile([P, 512], F32, tag="mm")
                    for co in range(NCO):
                        nc.tensor.matmul(
                            ps[:],
                            lhsT=w_sb[:, co, cb * P:(cb + 1) * P],
                            rhs=x_tile[:, co, :],
                            start=(co == 0), stop=(co == NCO - 1),
                        )
                    if cb == 0:
                        nc.vector.tensor_copy(qT_all[:, tg, :], ps[:])
                    elif cb == 1:
                        nc.vector.tensor_copy(kT_all[:, tg, :], ps[:])
                    else:
                        vt = vtp.tile([P, 512], F32R, tag="vt")
                        nc.scalar.copy(vt[:], ps[:])
                        for j in range(4):
                            kb = tg * 4 + j
                            pst = ps_mm.tile([P, P], F32, tag="mm")
                            nc.tensor.transpose(
                                pst[:], vt[:, j * P:(j + 1) * P].bitcast(F32),
                                ident[:],
                            )
                            for h in range(H_LOC):
                                nc.vector.tensor_copy(
                                    v_aug[h][:, kb, 0:DH],
                                    pst[:, h * DH:(h + 1) * DH],
                                )

            nc.sync.dma_start(wo_sb[:], wo[:])

            # ---- phase 2+3: attention + output projection per (b, qg) ----
            # Software-pipelined: group g's projection matmuls are emitted
            # after group g+1's attention matmuls, so the normalize chain
            # (DVE/GpSimd) overlaps PE work instead of stalling it. Inside
            # the k-loop, PV matmuls are delayed by one key block so the
            # exp (ScalarE) latency is hidden behind the next ST matmuls.

            def proj_step(tgq, yT, ti, cohalf):
                tok0 = tgq * 512
                po = ps_mm.tile([P, 512], F32, tag="mm", name="po")
                nc.tensor.matmul(
                    po[:],
                    lhsT=yT[:, ti * P:(ti + 1) * P],
                    rhs=wo_sb[:, cohalf * 512:(cohalf + 1) * 512],
                    start=True, stop=True,
                )
                ob = outp.tile([P, 512], F32, tag="ob", name="ob")
                if cohalf == 0:
                    nc.vector.tensor_copy(ob[:], po[:])
                else:
                    nc.scalar.copy(ob[:], po[:])
                nc.sync.dma_start(
                    out[tok0 + ti * P:tok0 + (ti + 1) * P,
                        cohalf * 512:(cohalf + 1) * 512],
                    ob[:],
                )

            proj_queue = []  # (tgq, yT, ti, cohalf) steps awaiting emission
            for b in range(B):
                for qg in range(QG_PER_B):
                    tgq = QG_PER_B * b + qg
                    nkj_total = (qg + 1) * 4
                    OT = [
                        ps_ot.tile([DH + 1, 512], F32, tag="ot", name=f"ot{_h}")
                        for _h in range(H_LOC)
                    ]
                    pending = []  # [(pts, kb, q_lo, idx)] PV delayed 2 blocks

                    def emit_pv(batch):
                        # per-head consecutive accumulation into the same
                        # PSUM bank pipelines much better than interleaved
                        # single-shot matmuls
                        for h in range(H_LOC):
                            for pts, kb_, q_lo_, idx_ in batch:
                                nc.tensor.matmul(
                                    OT[h][:, q_lo_:512],
                                    lhsT=v_aug[h][:, kb_, :],
                                    rhs=pts[h][:, q_lo_:512],
                                    start=(idx_ == 0),
                                    stop=(idx_ == nkj_total - 1),
                                )

                    idx = 0
                    for kg in range(qg + 1):
                        diag = kg == qg
                        tgk = QG_PER_B * b + kg
                        for kj in range(4):
                            kb = tgk * 4 + kj
                            q_lo = kj * P if diag else 0
                            pts = []
                            for h in range(H_LOC):
                                hs = slice(h * DH, (h + 1) * DH)
                                st = ps_st.tile(
                                    [P, 512], F32, tag="st", name="st"
                                )
                                nc.tensor.matmul(
                                    st[:, q_lo:512],
                                    lhsT=kT_all[hs, tgk, kj * P:(kj + 1) * P],
                                    rhs=qT_all[hs, tgq, q_lo:512],
                                    start=True, stop=True,
                                )
                                pt = ptp.tile([P, 512], F32R, tag="pt", name="pt")
                                nc.scalar.activation(
                                    pt[:, q_lo:512], st[:, q_lo:512], Exp,
                                    bias=0.0, scale=float(SCALE),
                                )
                                if diag:
                                    nc.vector.tensor_mul(
                                        pt[:, kj * P:(kj + 1) * P],
                                        pt[:, kj * P:(kj + 1) * P],
                                        tri01[:],
                                    )
                                pts.append(pt)
                            pending.append((pts, kb, q_lo, idx))
                            idx += 1
                            if len(pending) == 2 and idx < nkj_total:
                                emit_pv(pending)
                                pending = []
                            # interleave one projection matmul of the
                            # previous group between attention iterations
                            if idx >= 2 and proj_queue:
                                proj_step(*proj_queue.pop(0))
                    emit_pv(pending)
                    while proj_queue:
                        proj_step(*proj_queue.pop(0))

                    # epilogue: normalize y = O / den (den in OT row 64)
                    yT = epp.tile([P, 512], F32R, tag="yt")
                    for h in range(H_LOC):
                        den = epp.tile([1, 512], F32, tag="den", name="den")
                        with nc.named_scope(f"ep{tgq}_den{h}"):
                            nc.vector.tensor_copy(den[:], OT[h][DH:DH + 1, :])
                        rbr = epp.tile([DH, 512], F32, tag="rbr", name="rbr")
                        with nc.named_scope(f"ep{tgq}_bcast{h}"):
                            nc.gpsimd.partition_broadcast(rbr[:], den[:], channels=DH)
                        rb = epp.tile([DH, 512], F32, tag="rb", name="rb")
                        with nc.named_scope(f"ep{tgq}_recip{h}"):
                            nc.vector.reciprocal_approx_fast(rb[:], rbr[:])
                        with nc.named_scope(f"ep{tgq}_mult{h}"):
                            nc.vector.tensor_mul(
                                yT[h * DH:(h + 1) * DH, :], OT[h][0:DH, :], rb[:]
                            )
                    # previous group's projection interleaves with the next
                    # group's attention via proj_queue
                    proj_queue = [
                        (tgq, yT, ti, cohalf)
                        for ti in range(4) for cohalf in range(2)
                    ]

            while proj_queue:
                proj_step(*proj_queue.pop(0))

    nc.compile()
    return nc


def _install_ntff_hook():
    try:
        from antenv.axon_hooks import get_axon_ntff_profile_hook  # noqa: F401
        return
    except ImportError:
        pass
    try:
        import trn_agent_boot.trn_boot as tb
        hook = tb._ntff_profile_via_ctypes("/opt/axon/libaxon_pjrt.so")
        mod = types.ModuleType("antenv.axon_hooks")
        mod.get_axon_ntff_profile_hook = lambda: hook
        mod.set_axon_ntff_profile_hook = lambda h: None
        sys.modules["antenv.axon_hooks"] = mod
    except Exception:
        pass


_NC_CACHE = None
LAST_EXEC_NS = None
LAST_TRACE = None


def kernel(x, w_qkv, w_out, trace=False):
    global _NC_CACHE, LAST_EXEC_NS, LAST_TRACE
    if _NC_CACHE is None:
        _NC_CACHE = build_nc()
    nc = _NC_CACHE

    x = np.asarray(x, dtype=np.float32)
    w_qkv = np.asarray(w_qkv, dtype=np.float32)
    w_out = np.asarray(w_out, dtype=np.float32)

    xT = np.ascontiguousarray(x.reshape(BT, C).T)  # [C, BT]
    in_maps = []
    for core in range(N_CORES):
        h0 = core * H_LOC * DH  # first local channel
        ch = slice(h0, h0 + H_LOC * DH)
        # wq columns: [q chans | k chans | v chans] for the local heads
        wq_i = np.concatenate(
            [w_qkv[s * C:s * C + C, :][ch, :] for s in range(3)], axis=0
        )  # [384, C]
        in_maps.append({
            "xT": xT,
            "wq": np.ascontiguousarray(wq_i.T),            # [C, 384]
            "wo": np.ascontiguousarray(w_out[:, ch].T),    # [128, C]
        })

    if trace:
        _install_ntff_hook()
    res = run_bass_kernel_spmd(
        nc, in_maps, core_ids=list(range(N_CORES)), trace=trace
    )
    LAST_EXEC_NS = res.exec_time_ns
    kernel_globals = globals()
    kernel_globals['LAST_RESULT'] = res
    LAST_TRACE = (
        res.instructions_and_trace[1] if res.instructions_and_trace else None
    )

    acc = np.zeros((BT, C), dtype=np.float64)
    for core in range(N_CORES):
        acc += res.results[core]["out"]
    return acc.astype(np.float32).reshape(B, T, C)


# revision 24
# speedup vs baseline: 1.0394x; 1.0394x over previous
"""Causal self-attention (B=4, T=1024, C=1024, H=16) on 8 TRN2 NeuronCores.

Sharding: tensor-parallel over heads — 2 heads per core. x is replicated;
each core computes qkv for its heads, attention, and a partial output
projection (its heads' columns of w_out); the host sums the 8 partials.

Per-core dataflow (all matmuls fp32r):
  phase 1: qkvT[chan, tok] = wqkvT.T @ xT per 512-token group;
           q,k kept as [d, tok] (2 heads packed on 128 partitions);
           v PE-transposed to [tok, d] with a ones column appended.
  phase 2: per (batch, 512-query group, 128-key block):
           ST[key, query] = kT.T @ qT  (2 heads row-packed in the PE array)
           PT = exp(ST/8)  (causal: 512-block skipping; the diagonal
           128x128 block gets a multiplicative 0/1 triangular mask on PT)
           OT[d+1, query] += v_aug.T @ PT  (row 64 accumulates the softmax
           denominator via the ones column)
           normalize: y = OT[0:64] * broadcast(1/OT[64])
  phase 3: out[tok, :] = yT.T @ woT, DMA'd PSUM->HBM directly.
"""

import sys
import types

import numpy as np

import concourse.bacc as bacc
import concourse.mybir as mybir
import concourse.tile as tile
from concourse.bass_utils import run_bass_kernel_spmd
from concourse.masks import make_identity

F32 = mybir.dt.float32
F32R = mybir.dt.float32r
Exp = mybir.ActivationFunctionType.Exp

P = 128
B = 4
T = 1024
C = 1024
N_HEAD = 16
DH = 64
BT = B * T           # 4096 tokens
NCO = C // P         # 8 contraction blocks
NTG = BT // 512      # 8 token groups of 512
QG_PER_B = T // 512  # 2 query groups per batch
N_CORES = 8
H_LOC = N_HEAD // N_CORES  # 2 local heads

SCALE = 1.0 / np.sqrt(np.float32(DH))  # 0.125


def build_nc():
    nc = bacc.Bacc("TRN2", target_bir_lowering=False, debug=False)

    xT = nc.dram_tensor("xT", [C, BT], F32R, kind="ExternalInput")
    wq = nc.dram_tensor("wq", [C, 3 * P], F32R, kind="ExternalInput")
    wo = nc.dram_tensor("wo", [P, C], F32R, kind="ExternalInput")
    out = nc.dram_tensor("out", [BT, C], F32, kind="ExternalOutput")

    with tile.TileContext(nc) as tc:
        with (
            tc.tile_pool(name="consts", bufs=1) as consts,
            tc.tile_pool(name="xin", bufs=2) as xin,
            tc.tile_pool(name="vt", bufs=3) as vtp,
            tc.tile_pool(name="pt", bufs=6) as ptp,
            tc.tile_pool(name="ep", bufs=4) as epp,
            tc.tile_pool(name="outp", bufs=4) as outp,
            tc.tile_pool(name="ps_mm", bufs=2, space="PSUM") as ps_mm,
            tc.tile_pool(name="ps_st", bufs=4, space="PSUM") as ps_st,
            tc.tile_pool(name="ps_ot", bufs=2, space="PSUM") as ps_ot,
        ):
            # ---- constants / persistent tiles ----
            # first x tile's chunks go first so the PE can start ~asap;
            # weights interleave per contraction chunk
            x_first = xin.tile([P, NCO, 512], F32R, tag="x", name="x_first")
            w_sb = consts.tile([P, NCO, 3 * P], F32R)
            for co in range(NCO):
                nc.sync.dma_start(x_first[:, co, :], xT[co * P:(co + 1) * P, 0:512])
                nc.sync.dma_start(w_sb[:, co, :], wq[co * P:(co + 1) * P, :])
            wo_sb = consts.tile([P, C], F32R)

            qT_all = consts.tile([P, NTG, 512], F32R)
            kT_all = consts.tile([P, NTG, 512], F32R)
            # v with ones column at index DH (softmax denominator trick)
            v_aug = [
                consts.tile([P, BT // P, DH + 1], F32R, tag=f"v{h}", name=f"v{h}")
                for h in range(H_LOC)
            ]

            ident = consts.tile([P, P], F32)
            make_identity(nc, ident[:])
            # multiplicative causal mask for the diagonal 128x128 block of
            # PT[key, query]: valid iff key_row <= query_col (keep where
            # col - row >= 0). Applied to the exp output in SBUF, off the
            # ST-psum critical path.
            tri01 = consts.tile([P, P], F32)
            nc.gpsimd.memset(tri01[:], 1.0)
            nc.gpsimd.affine_select(
                out=tri01[:], in_=tri01[:],
                compare_op=mybir.AluOpType.is_ge, fill=0.0,
                base=0, pattern=[[1, P]], channel_multiplier=-1,
            )
            ones_f = consts.tile([P, BT // P], F32)
            nc.vector.memset(ones_f[:], 1.0)
            for h in range(H_LOC):
                nc.vector.tensor_copy(v_aug[h][:, :, DH], ones_f[:])

            # ---- phase 1: qkv projection per 512-token group ----
            for tg in range(NTG):
                if tg == 0:
                    x_tile = x_first
                else:
                    x_tile = xin.tile([P, NCO, 512], F32R, tag="x")
                    nc.sync.dma_start(
                        x_tile[:],
                        xT[:].rearrange("(a p) t -> p a t", p=P)[
                            :, :, tg * 512:(tg + 1) * 512
                        ],
                    )
                for cb in range(3):  # 0=q, 1=k, 2=v
                    ps = ps_mm.tile([P, 512], F32, tag="mm")
                    for co in range(NCO):
                        nc.tensor.matmul(
                            ps[:],
                            lhsT=w_sb[:, co, cb * P:(cb + 1) * P],
                            rhs=x_tile[:, co, :],
                            start=(co == 0), stop=(co == NCO - 1),
                        )
                    if cb == 0:
                        nc.vector.tensor_copy(qT_all[:, tg, :], ps[:])
                    elif cb == 1:
                        nc.vector.tensor_copy(kT_all[:, tg, :], ps[:])
                    else:
                        vt = vtp.tile([P, 512], F32R, tag="vt")
                        nc.scalar.copy(vt[:], ps[:])
                        for j in range(4):
                            kb = tg * 4 + j
                            pst = ps_mm.tile([P, P], F32, tag="mm")
                            nc.tensor.transpose(
                                pst[:], vt[:, j * P:(j + 1) * P].bitcast(F32),
                                ident[:],
                            )
                            for h in range(H_LOC):
                                nc.vector.tensor_copy(
                                    v_aug[h][:, kb, 0:DH],
                                    pst[:, h * DH:(h + 1) * DH],
                                )

            nc.sync.dma_start(wo_sb[:], wo[:])

            # ---- phase 2+3: attention + output projection per (b, qg) ----
            # Software-pipelined: group g's projection matmuls are emitted
            # after group g+1's attention matmuls, so the normalize chain
            # (DVE/GpSimd) overlaps PE work instead of stalling it. Inside
            # the k-loop, PV matmuls are delayed by one key block so the
            # exp (ScalarE) latency is hidden behind the next ST matmuls.

            def proj_step(tgq, yT, ti, cohalf):
                tok0 = tgq * 512
                po = ps_mm.tile([P, 512], F32, tag="mm", name="po")
                nc.tensor.matmul(
                    po[:],
                    lhsT=yT[:, ti * P:(ti + 1) * P],
                    rhs=wo_sb[:, cohalf * 512:(cohalf + 1) * 512],
                    start=True, stop=True,
                )
                ob = outp.tile([P, 512], F32, tag="ob", name="ob")
                if cohalf == 0:
                    nc.vector.tensor_copy(ob[:], po[:])
                else:
                    nc.scalar.copy(ob[:], po[:])
                nc.sync.dma_start(
                    out[tok0 + ti * P:tok0 + (ti + 1) * P,
                        cohalf * 512:(cohalf + 1) * 512],
                    ob[:],
                )

            proj_queue = []  # (tgq, yT, ti, cohalf) steps awaiting emission
            for b in range(B):
                for qg in range(QG_PER_B):
                    tgq = QG_PER_B * b + qg
                    nkj_total = (qg + 1) * 4
                    OT = [
                        ps_ot.tile([DH + 1, 512], F32, tag="ot", name=f"ot{_h}")
                        for _h in range(H_LOC)
                    ]
                    pending = []  # [(pts, kb, q_lo, idx)] PV delayed 2 blocks

                    def emit_pv(batch):
                        # per-head consecutive accumulation into the same
                        # PSUM bank pipelines much better than interleaved
                        # single-shot matmuls
                        for h in range(H_LOC):
                            for pts, kb_, q_lo_, idx_ in batch:
                                nc.tensor.matmul(
                                    OT[h][:, q_lo_:512],
                                    lhsT=v_aug[h][:, kb_, :],
                                    rhs=pts[h][:, q_lo_:512],
                                    start=(idx_ == 0),
                                    stop=(idx_ == nkj_total - 1),
                                )

                    idx = 0
                    for kg in range(qg + 1):
                        diag = kg == qg
                        tgk = QG_PER_B * b + kg
                        for kj in range(4):
                            kb = tgk * 4 + kj
                            q_lo = kj * P if diag else 0
                            pts = []
                            for h in range(H_LOC):
                                hs = slice(h * DH, (h + 1) * DH)
                                st = ps_st.tile(
                                    [P, 512], F32, tag="st", name="st"
                                )
                                nc.tensor.matmul(
                                    st[:, q_lo:512],
                                    lhsT=kT_all[hs, tgk, kj * P:(kj + 1) * P],
                                    rhs=qT_all[hs, tgq, q_lo:512],
                                    start=True, stop=True,
                                )
                                pt = ptp.tile([P, 512], F32R, tag="pt", name="pt")
                                nc.scalar.activation(
                                    pt[:, q_lo:512], st[:, q_lo:512], Exp,
                                    bias=0.0, scale=float(SCALE),
                                )
                                if diag:
                                    nc.vector.tensor_mul(
                                        pt[:, kj * P:(kj + 1) * P],
                                        pt[:, kj * P:(kj + 1) * P],
                                        tri01[:],
                                    )
                                pts.append(pt)
                            pending.append((pts, kb, q_lo, idx))
                            idx += 1
                            if len(pending) == 2 and idx < nkj_total:
                                emit_pv(pending)
                                pending = []
                            # interleave one projection matmul of the
                            # previous group between attention iterations
                            if idx >= 2 and proj_queue:
                                proj_step(*proj_queue.pop(0))
                    emit_pv(pending)
                    while proj_queue:
                        proj_step(*proj_queue.pop(0))

                    # epilogue: normalize y = O / den (den in OT row 64)
                    yT = epp.tile([P, 512], F32R, tag="yt")
                    for h in range(H_LOC):
                        den = epp.tile([1, 512], F32, tag="den", name="den")
                        with nc.named_scope(f"ep{tgq}_den{h}"):
                            nc.vector.tensor_copy(den[:], OT[h][DH:DH + 1, :])
                        rbr = epp.tile([DH, 512], F32, tag="rbr", name="rbr")
                        with nc.named_scope(f"ep{tgq}_bcast{h}"):
                            nc.gpsimd.partition_broadcast(rbr[:], den[:], channels=DH)
                        rb = epp.tile([DH, 512], F32, tag="rb", name="rb")
                        with nc.named_scope(f"ep{tgq}_recip{h}"):
                            nc.vector.reciprocal_approx_fast(rb[:], rbr[:])
                        with nc.named_scope(f"ep{tgq}_mult{h}"):
                            nc.vector.tensor_mul(
                                yT[h * DH:(h + 1) * DH, :], OT[h][0:DH, :], rb[:]
                            )
                    # previous group's projection interleaves with the next
                    # group's attention via proj_queue
                    proj_queue = [
                        (tgq, yT, ti, cohalf)
                        for ti in range(4) for cohalf in range(2)
                    ]

            while proj_queue:
                proj_step(*proj_queue.pop(0))

    nc.compile()
    return nc


def _install_ntff_hook():
    try:
        from antenv.axon_hooks import get_axon_ntff_profile_hook  # noqa: F401
        return
    except ImportError:
        pass
    try:
        import trn_agent_boot.trn_boot as tb
        hook = tb._ntff_profile_via_ctypes("/opt/axon/libaxon_pjrt.so")
        mod = types.ModuleType("antenv.axon_hooks")
        mod.get_axon_ntff_profile_hook = lambda: hook
        mod.set_axon_ntff_profile_hook = lambda h: None
        sys.modules["antenv.axon_hooks"] = mod
    except Exception:
        pass


_NC_CACHE = None
LAST_EXEC_NS = None
LAST_TRACE = None


def kernel(x, w_qkv, w_out, trace=False):
    global _NC_CACHE, LAST_EXEC_NS, LAST_TRACE
    if _NC_CACHE is None:
        _NC_CACHE = build_nc()
    nc = _NC_CACHE

    x = np.asarray(x, dtype=np.float32)
    w_qkv = np.asarray(w_qkv, dtype=np.float32)
    w_out = np.asarray(w_out, dtype=np.float32)

    xT = np.ascontiguousarray(x.reshape(BT, C).T)  # [C, BT]
    in_maps = []
    for core in range(N_CORES):
        h0 = core * H_LOC * DH  # first local channel
        ch = slice(h0, h0 + H_LOC * DH)
        # wq columns: [q chans | k chans | v chans] for the local heads
        wq_i = np.concatenate(
            [w_qkv[s * C:s * C + C, :][ch, :] for s in range(3)], axis=0
        )  # [384, C]
        in_maps.append({
            "xT": xT,
            "wq": np.ascontiguousarray(wq_i.T),            # [C, 384]
            "wo": np.ascontiguousarray(w_out[:, ch].T),    # [128, C]
        })

    if trace:
        _install_ntff_hook()
    res = run_bass_kernel_spmd(
        nc, in_maps, core_ids=list(range(N_CORES)), trace=trace
    )
    LAST_EXEC_NS = res.exec_time_ns
    kernel_globals = globals()
    kernel_globals['LAST_RESULT'] = res
    LAST_TRACE = (
        res.instructions_and_trace[1] if res.instructions_and_trace else None
    )

    acc = np.zeros((BT, C), dtype=np.float64)
    for core in range(N_CORES):
        acc += res.results[core]["out"]
    return acc.astype(np.float32).reshape(B, T, C)
